# revision 1
# baseline (speedup 1.0000x reference)
"""Trainium2 Bass kernel for the vq_codebook problem (2-layer LSTM + VQ).

Self-contained: accepts FULL inputs (B=2048), shards batch across 8 NeuronCores,
runs a Bass/Tile LSTM+VQ kernel per core, and reassembles the full output on host.

Computation layout (per core, B_c = 256):
  - LSTM runs "transposed": state tiles are [128 partitions = hidden-channel,
    free = j*256 + b] with j in {0,1} covering H=256 channels.
  - Gate pre-activations accumulate in PSUM as [128, 8*256] with gate channels
    permuted to (i, f, o, g) order so sigmoid(i|f|o) is a single ACT op.
  - Matmul operands are fp16 (PE streams 16-bit at full rate; fp32 accumulate),
    elementwise state math is fp32.
  - Biases ride the matmuls: layer0 via a ones-row appended to the x tile,
    layer1 via a K=1 matmul against a ones row.
  - VQ tail emits per-core segment sums / counts / sum-of-squares; the host
    all-reduces those (exactly the distributed all-reduce the sharding hint
    calls for) and finishes the centroid update + scalar loss.
"""

import numpy as np

import concourse.bass as bass
import concourse.mybir as mybir
import concourse.tile as tile
from concourse import bacc
from concourse.bass_utils import run_bass_kernel_spmd
from concourse.masks import make_identity

F16 = mybir.dt.float16
F32 = mybir.dt.float32
AF = mybir.ActivationFunctionType
ALU = mybir.AluOpType
AX = mybir.AxisListType

L = 200
B_TOTAL = 2048
N_CORES = 8
BC = B_TOTAL // N_CORES  # 256
HIN = 64
H = 256
G4 = 4 * H  # 1024
KC = 64  # clusters
BETA = 0.1

# gate order in torch reference: i, f, g, o (each H rows). We permute to i, f, o, g.
_PERM = np.concatenate(
    [np.arange(0, 2 * H), np.arange(3 * H, 4 * H), np.arange(2 * H, 3 * H)]
)


def _build(T=L):
    nc = bacc.Bacc("TRN2", target_bir_lowering=False, debug=False)

    xt_d = nc.dram_tensor("xt", [T, HIN + 1, BC], F16, kind="ExternalInput")
    w0k01_d = nc.dram_tensor("w0k01", [128, 2 * G4], F16, kind="ExternalInput")
    w0k2_d = nc.dram_tensor("w0k2", [HIN + 1, G4], F16, kind="ExternalInput")
    w1k_d = nc.dram_tensor("w1k", [128, 4 * G4], F16, kind="ExternalInput")
    b1r_d = nc.dram_tensor("b1r", [1, G4], F16, kind="ExternalInput")
    c2t_d = nc.dram_tensor("c2t", [128, 2 * KC], F32, kind="ExternalInput")
    cnorm_d = nc.dram_tensor("cnorm", [128, KC], F32, kind="ExternalInput")

    lat_d = nc.dram_tensor("lat", [BC, H], F32, kind="ExternalOutput")
    seg_d = nc.dram_tensor("seg", [KC, H + 2], F32, kind="ExternalOutput")

    with tile.TileContext(nc) as tc:
        with (
            tc.tile_pool(name="consts", bufs=1) as consts,
            tc.tile_pool(name="xtp", bufs=8) as xtp,
            tc.tile_pool(name="hp", bufs=3) as hp,
            tc.tile_pool(name="cp", bufs=2) as cp,
            tc.tile_pool(name="sifop", bufs=3) as sifop,
            tc.tile_pool(name="ewp", bufs=3) as ewp,
            tc.tile_pool(name="vqsb", bufs=2) as vqsb,
        ):
            # ---- constants / weights to SBUF ----
            w0k01 = consts.tile([128, 2 * G4], F16, tag="w0k01")
            nc.sync.dma_start(w0k01[:], w0k01_d[:, :])
            w0k2 = consts.tile([HIN + 1, G4], F16, tag="w0k2")
            nc.sync.dma_start(w0k2[:], w0k2_d[:, :])
            w1k = consts.tile([128, 4 * G4], F16, tag="w1k")
            nc.sync.dma_start(w1k[:], w1k_d[:, :])
            b1r = consts.tile([1, G4], F16, tag="b1r")
            nc.sync.dma_start(b1r[:], b1r_d[:, :])
            c2t = consts.tile([128, 2 * KC], F32, tag="c2t")
            nc.sync.dma_start(c2t[:], c2t_d[:, :])
            cnorm = consts.tile([128, KC], F32, tag="cnorm")
            nc.sync.dma_start(cnorm[:], cnorm_d[:, :])
            ones = consts.tile([1, BC], F16, tag="ones")
            nc.vector.memset(ones[:], 1.0)
            ident = consts.tile([128, 128], F32, tag="ident")
            make_identity(nc, ident[:])
            lat_sb = consts.tile([128, 2 * BC], F32, tag="lat_sb")

            # ---- initial states ----
            h0 = hp.tile([128, 2 * BC], F16, tag="h0")
            h1 = hp.tile([128, 2 * BC], F16, tag="h1")
            c0 = cp.tile([128, 2 * BC], F32, tag="c0")
            c1 = cp.tile([128, 2 * BC], F32, tag="c1")
            nc.vector.memset(h0[:], 0.0)
            nc.vector.memset(h1[:], 0.0)
            nc.vector.memset(c0[:], 0.0)
            nc.vector.memset(c1[:], 0.0)

            with (
                tc.tile_pool(name="g0psum", bufs=1, space="PSUM") as g0p,
                tc.tile_pool(name="g1psum", bufs=1, space="PSUM") as g1p,
            ):

                def ew_layer(g_ps, c_prev, h_out_f16, layer, also_f32=None):
                    """gate psum [128, 2048] (i|f|o|g blocks of 512) -> new c (f32), h."""
                    sig = sifop.tile([128, 3 * 2 * BC], F32, tag="sig")
                    nc.scalar.activation(sig[:], g_ps[:, 0 : 3 * 2 * BC], AF.Sigmoid)
                    tg = ewp.tile([128, 2 * BC], F32, tag="tg")
                    nc.scalar.activation(tg[:], g_ps[:, 3 * 2 * BC : 4 * 2 * BC], AF.Tanh)
                    fc = ewp.tile([128, 2 * BC], F32, tag="fc")
                    nc.vector.tensor_tensor(
                        fc[:], sig[:, 2 * BC : 4 * BC], c_prev[:], ALU.mult
                    )
                    ig = ewp.tile([128, 2 * BC], F32, tag="ig")
                    nc.vector.tensor_tensor(ig[:], sig[:, 0 : 2 * BC], tg[:], ALU.mult)
                    c_new = cp.tile([128, 2 * BC], F32, tag=f"c{layer}")
                    nc.vector.tensor_tensor(c_new[:], fc[:], ig[:], ALU.add)
                    tc_t = ewp.tile([128, 2 * BC], F32, tag="tc")
                    nc.scalar.activation(tc_t[:], c_new[:], AF.Tanh)
                    nc.vector.tensor_tensor(
                        h_out_f16[:], sig[:, 4 * BC : 6 * BC], tc_t[:], ALU.mult
                    )
                    if also_f32 is not None:
                        nc.vector.tensor_tensor(
                            also_f32[:], sig[:, 4 * BC : 6 * BC], tc_t[:], ALU.mult
                        )
                    return c_new

                for t in range(T):
                    xt = xtp.tile([HIN + 1, BC], F16, tag="xt")
                    nc.sync.dma_start(xt[:], xt_d[t])

                    # ---- layer 0 matmuls: gates0 = W_hh0 @ h0 + [W_ih0; b0] @ [x; 1]
                    g0 = g0p.tile([128, 8 * BC], F32, tag="g0")
                    for m in range(8):
                        ps = g0[:, BC * m : BC * (m + 1)]
                        nc.tensor.matmul(
                            ps,
                            w0k01[:, 128 * m : 128 * m + 128],
                            h0[:, 0:BC],
                            start=True,
                            stop=False,
                        )
                        nc.tensor.matmul(
                            ps,
                            w0k01[:, G4 + 128 * m : G4 + 128 * m + 128],
                            h0[:, BC : 2 * BC],
                            start=False,
                            stop=False,
                        )
                        nc.tensor.matmul(
                            ps,
                            w0k2[:, 128 * m : 128 * m + 128],
                            xt[:],
                            start=False,
                            stop=True,
                        )

                    h0n = hp.tile([128, 2 * BC], F16, tag="h0")
                    c0 = ew_layer(g0, c0, h0n, 0)

                    # ---- layer 1 matmuls: gates1 = b1 + W_hh1 @ h1 + W_ih1 @ h0n
                    g1 = g1p.tile([128, 8 * BC], F32, tag="g1")
                    for m in range(8):
                        ps = g1[:, BC * m : BC * (m + 1)]
                        nc.tensor.matmul(
                            ps,
                            b1r[0:1, 128 * m : 128 * m + 128],
                            ones[0:1, :],
                            start=True,
                            stop=False,
                        )
                        for k, mov in (
                            (0, h1[:, 0:BC]),
                            (1, h1[:, BC : 2 * BC]),
                            (2, h0n[:, 0:BC]),
                            (3, h0n[:, BC : 2 * BC]),
                        ):
                            nc.tensor.matmul(
                                ps,
                                w1k[:, G4 * k + 128 * m : G4 * k + 128 * m + 128],
                                mov,
                                start=False,
                                stop=(k == 3),
                            )

                    h1n = hp.tile([128, 2 * BC], F16, tag="h1")
                    c1 = ew_layer(
                        g1, c1, h1n, 1, also_f32=lat_sb if t == T - 1 else None
                    )
                    h0, h1 = h0n, h1n

            # ---- VQ tail ----
            with (
                tc.tile_pool(name="vqps", bufs=2, space="PSUM") as vqps,
                tc.tile_pool(name="segps", bufs=1, space="PSUM") as segps,
            ):
                seg_ps = segps.tile([KC, H + 2], F32, tag="segp")
                latb_tiles = []
                for b in range(2):
                    tp = vqps.tile([128, H], F32, tag="tp")
                    for j in range(2):
                        nc.tensor.transpose(
                            tp[:, 128 * j : 128 * j + 128],
                            lat_sb[:, BC * j + 128 * b : BC * j + 128 * b + 128],
                            ident[:],
                        )
                    latb = vqsb.tile([128, H + 2], F32, tag="latb")
                    latb_tiles.append(latb)
                    nc.vector.tensor_copy(latb[:, 0:H], tp[:])
                    nc.vector.memset(latb[:, H : H + 1], 1.0)
                    sq = vqsb.tile([128, H], F32, tag="sq")
                    nc.scalar.activation(
                        sq[:], tp[:], AF.Square, accum_out=latb[:, H + 1 : H + 2]
                    )
                    # D = lat @ (-2 C^T)  (+ ||c||^2)
                    dp = vqps.tile([128, KC], F32, tag="dp")
                    for j in range(2):
                        nc.tensor.matmul(
                            dp[:],
                            lat_sb[:, BC * j + 128 * b : BC * j + 128 * b + 128],
                            c2t[:, KC * j : KC * (j + 1)],
                            start=(j == 0),
                            stop=(j == 1),
                        )
                    dsb = vqsb.tile([128, KC], F32, tag="dsb")
                    nc.vector.tensor_tensor(dsb[:], dp[:], cnorm[:], ALU.add)
                    minv = vqsb.tile([128, 1], F32, tag="minv")
                    nc.vector.tensor_reduce(minv[:], dsb[:], axis=AX.X, op=ALU.min)
                    oh = vqsb.tile([128, KC], F32, tag="oh")
                    nc.vector.tensor_scalar(
                        oh[:], dsb[:], minv[:], None, op0=ALU.is_equal
                    )
                    nc.tensor.matmul(
                        seg_ps[:], oh[:], latb[:], start=(b == 0), stop=(b == 1)
                    )

                for b in range(2):
                    nc.sync.dma_start(
                        lat_d[128 * b : 128 * (b + 1), :], latb_tiles[b][:, 0:H]
                    )
                seg_sb = vqsb.tile([KC, H + 2], F32, tag="segsb")
                nc.vector.tensor_copy(seg_sb[:], seg_ps[:])
                nc.sync.dma_start(seg_d[:, :], seg_sb[:])

    nc.compile()
    return nc


_NC_CACHE = {}


def _get_nc(T=L):
    if T not in _NC_CACHE:
        _NC_CACHE[T] = _build(T)
    return _NC_CACHE[T]


def _prep_shared(W_ih0, W_hh0, b_ih0, b_hh0, W_ih1, W_hh1, b_ih1, b_hh1, clusters):
    p = _PERM
    w0k01 = (
        W_hh0[p].T.reshape(2, 128, G4).transpose(1, 0, 2).reshape(128, 2 * G4)
    ).astype(np.float16)
    w0k2 = np.concatenate(
        [W_ih0[p].T, (b_ih0 + b_hh0)[p][None, :]], axis=0
    ).astype(np.float16)
    wcat1 = np.concatenate([W_hh1[p].T, W_ih1[p].T], axis=0)  # (512, 1024)
    w1k = (
        wcat1.reshape(4, 128, G4).transpose(1, 0, 2).reshape(128, 4 * G4)
    ).astype(np.float16)
    b1r = (b_ih1 + b_hh1)[p][None, :].astype(np.float16)
    c2t = (
        (-2.0 * clusters).T.reshape(2, 128, KC).transpose(1, 0, 2).reshape(128, 2 * KC)
    ).astype(np.float32)
    cnorm = np.ascontiguousarray(
        np.broadcast_to((clusters * clusters).sum(1)[None, :], (128, KC))
    ).astype(np.float32)
    return dict(w0k01=w0k01, w0k2=w0k2, w1k=w1k, b1r=b1r, c2t=c2t, cnorm=cnorm)


def _prep_xt(X, c, T=L):
    xc = X[:T, c * BC : (c + 1) * BC, :]  # (T, BC, HIN)
    xt = np.empty((T, HIN + 1, BC), np.float16)
    xt[:, :HIN, :] = xc.transpose(0, 2, 1)
    xt[:, HIN, :] = 1.0
    return xt


def kernel(
    X,
    W_ih0,
    W_hh0,
    b_ih0,
    b_hh0,
    W_ih1,
    W_hh1,
    b_ih1,
    b_hh1,
    clusters,
    _T=L,
    _trace=False,
):
    nc = _get_nc(_T)
    shared = _prep_shared(
        W_ih0, W_hh0, b_ih0, b_hh0, W_ih1, W_hh1, b_ih1, b_hh1, clusters
    )
    in_maps = [dict(shared, xt=_prep_xt(X, c, _T)) for c in range(N_CORES)]
    res = run_bass_kernel_spmd(
        nc, in_maps, core_ids=list(range(N_CORES)), trace=_trace
    )

    latent = np.concatenate([r["lat"] for r in res.results], axis=0)
    seg = np.sum([r["seg"].astype(np.float64) for r in res.results], axis=0)

    sums = seg[:, :H]
    counts = seg[:, H]
    sumsq = seg[:, H + 1].sum()
    cl64 = clusters.astype(np.float64)
    upd = np.where(
        counts[:, None] > 0, sums / np.maximum(counts, 1.0)[:, None], cl64
    )
    loss = 0.5 * BETA * (
        sumsq - 2.0 * np.sum(upd * sums) + np.sum(counts * np.sum(upd * upd, axis=1))
    )
    out_loss = np.float32(loss)
    if _trace:
        kernel._last_results = res
    return latent, out_loss


# revision 4
# speedup vs baseline: 5.6977x; 5.6977x over previous
"""Trainium2 Bass kernel for the vq_codebook problem (2-layer LSTM + VQ).

Self-contained: accepts FULL inputs (B=2048), shards batch across 8 NeuronCores,
runs a Bass/Tile LSTM+VQ kernel per core, and reassembles the full output on host.

Computation layout (per core, B_c = 256):
  - LSTM runs "transposed": state tiles are [128 partitions = hidden-channel,
    free = j*256 + b] with j in {0,1} covering H=256 channels.
  - Gate pre-activations accumulate in PSUM as [128, 8*256] with gate channels
    permuted to (i, f, o, g) order so sigmoid(i|f|o) is a single ACT op.
  - Matmul operands are fp16 (PE streams 16-bit at full rate; fp32 accumulate),
    elementwise state math is fp32.
  - Biases ride the matmuls: layer0 via a ones-row appended to the x tile,
    layer1 via a K=1 matmul against a ones row.
  - VQ tail emits per-core segment sums / counts / sum-of-squares; the host
    all-reduces those (exactly the distributed all-reduce the sharding hint
    calls for) and finishes the centroid update + scalar loss.
"""

import numpy as np

import concourse.bass as bass
import concourse.mybir as mybir
import concourse.tile as tile
from concourse import bacc
from concourse.bass_utils import run_bass_kernel_spmd
from concourse.masks import make_identity

F16 = mybir.dt.float16
F32 = mybir.dt.float32
AF = mybir.ActivationFunctionType
ALU = mybir.AluOpType
AX = mybir.AxisListType

L = 200
B_TOTAL = 2048
N_CORES = 8
BC = B_TOTAL // N_CORES  # 256
HIN = 64
H = 256
G4 = 4 * H  # 1024
KC = 64  # clusters
BETA = 0.1

# gate order in torch reference: i, f, g, o (each H rows). We permute to i, f, o, g.
_PERM = np.concatenate(
    [np.arange(0, 2 * H), np.arange(3 * H, 4 * H), np.arange(2 * H, 3 * H)]
)


def _build(T=L, timing=False):
    nc = bacc.Bacc("TRN2", target_bir_lowering=False, debug=False)

    xt_d = nc.dram_tensor(
        "xt", [1 if timing else T, HIN + 1, BC], F16, kind="ExternalInput"
    )
    if not timing:
        w0k01_d = nc.dram_tensor("w0k01", [128, 2 * G4], F16, kind="ExternalInput")
        w0k2_d = nc.dram_tensor("w0k2", [HIN + 1, G4], F16, kind="ExternalInput")
        w1k_d = nc.dram_tensor("w1k", [128, 4 * G4], F16, kind="ExternalInput")
        b1r_d = nc.dram_tensor("b1r", [1, G4], F16, kind="ExternalInput")
        c2t_d = nc.dram_tensor("c2t", [128, 2 * KC], F32, kind="ExternalInput")
        cnorm_d = nc.dram_tensor("cnorm", [128, KC], F32, kind="ExternalInput")

    lat_d = nc.dram_tensor("lat", [BC, H], F32, kind="ExternalOutput")
    seg_d = nc.dram_tensor("seg", [KC, H + 2], F32, kind="ExternalOutput")

    with tile.TileContext(nc) as tc:
        with (
            tc.tile_pool(name="consts", bufs=1) as consts,
            tc.tile_pool(name="xtp", bufs=8) as xtp,
            tc.tile_pool(name="hp", bufs=3) as hp,
            tc.tile_pool(name="cp", bufs=2) as cp,
            tc.tile_pool(name="sifop", bufs=3) as sifop,
            tc.tile_pool(name="ewp", bufs=3) as ewp,
            tc.tile_pool(name="vqsb", bufs=2) as vqsb,
        ):
            # ---- constants / weights to SBUF ----
            w0k01 = consts.tile([128, 2 * G4], F16, tag="w0k01")
            w0k2 = consts.tile([HIN + 1, G4], F16, tag="w0k2")
            w1k = consts.tile([128, 4 * G4], F16, tag="w1k")
            b1r = consts.tile([1, G4], F16, tag="b1r")
            c2t = consts.tile([128, 2 * KC], F32, tag="c2t")
            cnorm = consts.tile([128, KC], F32, tag="cnorm")
            if timing:
                for tt in (w0k01, w0k2, w1k, b1r, c2t, cnorm):
                    nc.vector.memset(tt[:], 0.01)
            else:
                nc.sync.dma_start(w0k01[:], w0k01_d[:, :])
                nc.sync.dma_start(w0k2[:], w0k2_d[:, :])
                nc.sync.dma_start(w1k[:], w1k_d[:, :])
                nc.sync.dma_start(b1r[:], b1r_d[:, :])
                nc.sync.dma_start(c2t[:], c2t_d[:, :])
                nc.sync.dma_start(cnorm[:], cnorm_d[:, :])
            ones = consts.tile([1, BC], F16, tag="ones")
            nc.vector.memset(ones[:], 1.0)
            ident = consts.tile([128, 128], F32, tag="ident")
            make_identity(nc, ident[:])
            lat_sb = consts.tile([128, 2 * BC], F32, tag="lat_sb")

            # ---- initial states ----
            h0 = hp.tile([128, 2 * BC], F16, tag="h0")
            h1 = hp.tile([128, 2 * BC], F16, tag="h1")
            c0 = cp.tile([128, 2 * BC], F32, tag="c0")
            c1 = cp.tile([128, 2 * BC], F32, tag="c1")
            nc.vector.memset(h0[:], 0.0)
            nc.vector.memset(h1[:], 0.0)
            nc.vector.memset(c0[:], 0.0)
            nc.vector.memset(c1[:], 0.0)

            with (
                tc.tile_pool(name="g0psum", bufs=1, space="PSUM") as g0p,
                tc.tile_pool(name="g1psum", bufs=1, space="PSUM") as g1p,
            ):

                def ew_layer(g_ps, c_prev, h_out_f16, layer, also_f32=None):
                    """gate psum [128, 2048] (i|f|o|g blocks of 512) -> new c (f32), h."""
                    sig = sifop.tile([128, 3 * 2 * BC], F32, tag="sig")
                    nc.scalar.activation(sig[:], g_ps[:, 0 : 3 * 2 * BC], AF.Sigmoid)
                    tg = ewp.tile([128, 2 * BC], F32, tag="tg")
                    nc.scalar.activation(tg[:], g_ps[:, 3 * 2 * BC : 4 * 2 * BC], AF.Tanh)
                    fc = ewp.tile([128, 2 * BC], F32, tag="fc")
                    nc.vector.tensor_tensor(
                        fc[:], sig[:, 2 * BC : 4 * BC], c_prev[:], ALU.mult
                    )
                    ig = ewp.tile([128, 2 * BC], F32, tag="ig")
                    nc.vector.tensor_tensor(ig[:], sig[:, 0 : 2 * BC], tg[:], ALU.mult)
                    c_new = cp.tile([128, 2 * BC], F32, tag=f"c{layer}")
                    nc.vector.tensor_tensor(c_new[:], fc[:], ig[:], ALU.add)
                    tc_t = ewp.tile([128, 2 * BC], F32, tag="tc")
                    nc.scalar.activation(tc_t[:], c_new[:], AF.Tanh)
                    nc.vector.tensor_tensor(
                        h_out_f16[:], sig[:, 4 * BC : 6 * BC], tc_t[:], ALU.mult
                    )
                    if also_f32 is not None:
                        nc.vector.tensor_tensor(
                            also_f32[:], sig[:, 4 * BC : 6 * BC], tc_t[:], ALU.mult
                        )
                    return c_new

                for t in range(T):
                    xt = xtp.tile([HIN + 1, BC], F16, tag="xt")
                    nc.sync.dma_start(xt[:], xt_d[0 if timing else t])

                    # ---- layer 0 matmuls: gates0 = W_hh0 @ h0 + [W_ih0; b0] @ [x; 1]
                    g0 = g0p.tile([128, 8 * BC], F32, tag="g0")
                    for m in range(8):
                        ps = g0[:, BC * m : BC * (m + 1)]
                        nc.tensor.matmul(
                            ps,
                            w0k01[:, 128 * m : 128 * m + 128],
                            h0[:, 0:BC],
                            start=True,
                            stop=False,
                        )
                        nc.tensor.matmul(
                            ps,
                            w0k01[:, G4 + 128 * m : G4 + 128 * m + 128],
                            h0[:, BC : 2 * BC],
                            start=False,
                            stop=False,
                        )
                        nc.tensor.matmul(
                            ps,
                            w0k2[:, 128 * m : 128 * m + 128],
                            xt[:],
                            start=False,
                            stop=True,
                        )

                    h0n = hp.tile([128, 2 * BC], F16, tag="h0")
                    c0 = ew_layer(g0, c0, h0n, 0)

                    # ---- layer 1 matmuls: gates1 = b1 + W_hh1 @ h1 + W_ih1 @ h0n
                    g1 = g1p.tile([128, 8 * BC], F32, tag="g1")
                    for m in range(8):
                        ps = g1[:, BC * m : BC * (m + 1)]
                        nc.tensor.matmul(
                            ps,
                            b1r[0:1, 128 * m : 128 * m + 128],
                            ones[0:1, :],
                            start=True,
                            stop=False,
                        )
                        for k, mov in (
                            (0, h1[:, 0:BC]),
                            (1, h1[:, BC : 2 * BC]),
                            (2, h0n[:, 0:BC]),
                            (3, h0n[:, BC : 2 * BC]),
                        ):
                            nc.tensor.matmul(
                                ps,
                                w1k[:, G4 * k + 128 * m : G4 * k + 128 * m + 128],
                                mov,
                                start=False,
                                stop=(k == 3),
                            )

                    h1n = hp.tile([128, 2 * BC], F16, tag="h1")
                    c1 = ew_layer(
                        g1, c1, h1n, 1, also_f32=lat_sb if t == T - 1 else None
                    )
                    h0, h1 = h0n, h1n

            # ---- VQ tail ----
            with (
                tc.tile_pool(name="vqps", bufs=2, space="PSUM") as vqps,
                tc.tile_pool(name="segps", bufs=1, space="PSUM") as segps,
            ):
                seg_ps = segps.tile([KC, H + 2], F32, tag="segp")
                latb_tiles = []
                for b in range(2):
                    tp = vqps.tile([128, H], F32, tag="tp")
                    for j in range(2):
                        nc.tensor.transpose(
                            tp[:, 128 * j : 128 * j + 128],
                            lat_sb[:, BC * j + 128 * b : BC * j + 128 * b + 128],
                            ident[:],
                        )
                    latb = vqsb.tile([128, H + 2], F32, tag="latb")
                    latb_tiles.append(latb)
                    nc.vector.tensor_copy(latb[:, 0:H], tp[:])
                    nc.vector.memset(latb[:, H : H + 1], 1.0)
                    sq = vqsb.tile([128, H], F32, tag="sq")
                    nc.scalar.activation(
                        sq[:], tp[:], AF.Square, accum_out=latb[:, H + 1 : H + 2]
                    )
                    # D = lat @ (-2 C^T)  (+ ||c||^2)
                    dp = vqps.tile([128, KC], F32, tag="dp")
                    for j in range(2):
                        nc.tensor.matmul(
                            dp[:],
                            lat_sb[:, BC * j + 128 * b : BC * j + 128 * b + 128],
                            c2t[:, KC * j : KC * (j + 1)],
                            start=(j == 0),
                            stop=(j == 1),
                        )
                    dsb = vqsb.tile([128, KC], F32, tag="dsb")
                    nc.vector.tensor_tensor(dsb[:], dp[:], cnorm[:], ALU.add)
                    minv = vqsb.tile([128, 1], F32, tag="minv")
                    nc.vector.tensor_reduce(minv[:], dsb[:], axis=AX.X, op=ALU.min)
                    oh = vqsb.tile([128, KC], F32, tag="oh")
                    nc.vector.tensor_scalar(
                        oh[:], dsb[:], minv[:], None, op0=ALU.is_equal
                    )
                    nc.tensor.matmul(
                        seg_ps[:], oh[:], latb[:], start=(b == 0), stop=(b == 1)
                    )

                for b in range(2):
                    nc.sync.dma_start(
                        lat_d[128 * b : 128 * (b + 1), :], latb_tiles[b][:, 0:H]
                    )
                seg_sb = vqsb.tile([KC, H + 2], F32, tag="segsb")
                nc.vector.tensor_copy(seg_sb[:], seg_ps[:])
                nc.sync.dma_start(seg_d[:, :], seg_sb[:])

    nc.compile()
    return nc


_NC_CACHE = {}


def _get_nc(T=L):
    if T not in _NC_CACHE:
        _NC_CACHE[T] = _build(T)
    return _NC_CACHE[T]


def _prep_shared(W_ih0, W_hh0, b_ih0, b_hh0, W_ih1, W_hh1, b_ih1, b_hh1, clusters):
    p = _PERM
    w0k01 = (
        W_hh0[p].T.reshape(2, 128, G4).transpose(1, 0, 2).reshape(128, 2 * G4)
    ).astype(np.float16)
    w0k2 = np.concatenate(
        [W_ih0[p].T, (b_ih0 + b_hh0)[p][None, :]], axis=0
    ).astype(np.float16)
    wcat1 = np.concatenate([W_hh1[p].T, W_ih1[p].T], axis=0)  # (512, 1024)
    w1k = (
        wcat1.reshape(4, 128, G4).transpose(1, 0, 2).reshape(128, 4 * G4)
    ).astype(np.float16)
    b1r = (b_ih1 + b_hh1)[p][None, :].astype(np.float16)
    c2t = (
        (-2.0 * clusters).T.reshape(2, 128, KC).transpose(1, 0, 2).reshape(128, 2 * KC)
    ).astype(np.float32)
    cnorm = np.ascontiguousarray(
        np.broadcast_to((clusters * clusters).sum(1)[None, :], (128, KC))
    ).astype(np.float32)
    return dict(w0k01=w0k01, w0k2=w0k2, w1k=w1k, b1r=b1r, c2t=c2t, cnorm=cnorm)


def _prep_xt(X, c, T=L):
    xc = X[:T, c * BC : (c + 1) * BC, :]  # (T, BC, HIN)
    xt = np.empty((T, HIN + 1, BC), np.float16)
    xt[:, :HIN, :] = xc.transpose(0, 2, 1)
    xt[:, HIN, :] = 1.0
    return xt


def kernel(
    X,
    W_ih0,
    W_hh0,
    b_ih0,
    b_hh0,
    W_ih1,
    W_hh1,
    b_ih1,
    b_hh1,
    clusters,
    _T=L,
    _trace=False,
):
    nc = _get_nc(_T)
    shared = _prep_shared(
        W_ih0, W_hh0, b_ih0, b_hh0, W_ih1, W_hh1, b_ih1, b_hh1, clusters
    )
    in_maps = [dict(shared, xt=_prep_xt(X, c, _T)) for c in range(N_CORES)]
    res = run_bass_kernel_spmd(
        nc, in_maps, core_ids=list(range(N_CORES)), trace=_trace
    )

    latent = np.concatenate([r["lat"] for r in res.results], axis=0)
    seg = np.sum([r["seg"].astype(np.float64) for r in res.results], axis=0)

    sums = seg[:, :H]
    counts = seg[:, H]
    sumsq = seg[:, H + 1].sum()
    cl64 = clusters.astype(np.float64)
    upd = np.where(
        counts[:, None] > 0, sums / np.maximum(counts, 1.0)[:, None], cl64
    )
    loss = 0.5 * BETA * (
        sumsq - 2.0 * np.sum(upd * sums) + np.sum(counts * np.sum(upd * upd, axis=1))
    )
    out_loss = np.float32(loss)
    if _trace:
        kernel._last_results = res
    return latent, out_loss


# revision 7
# speedup vs baseline: 10.8605x; 1.9061x over previous
"""Trainium2 Bass kernel for the vq_codebook problem (2-layer LSTM + VQ).

Self-contained: accepts FULL inputs (B=2048), shards batch across 8 NeuronCores,
runs a Bass/Tile LSTM+VQ kernel per core, and reassembles the full output on host.

Computation layout (per core, B_c = 256):
  - LSTM runs "transposed": state tiles are [128 partitions = hidden-channel,
    free = j*256 + b] with j in {0,1} covering H=256 channels.
  - Gate pre-activations accumulate in PSUM as [128, 8*256] with gate channels
    permuted to (i, f, o, g) order so sigmoid(i|f|o) is a single ACT op.
  - Matmul operands are fp16 (PE streams 16-bit at full rate; fp32 accumulate),
    elementwise state math is fp32.
  - Biases ride the matmuls: layer0 via a ones-row appended to the x tile,
    layer1 via a K=1 matmul against a ones row.
  - VQ tail emits per-core segment sums / counts / sum-of-squares; the host
    all-reduces those (exactly the distributed all-reduce the sharding hint
    calls for) and finishes the centroid update + scalar loss.
"""

import numpy as np

import concourse.bass as bass
import concourse.mybir as mybir
import concourse.tile as tile
from concourse import bacc
from concourse.bass_utils import run_bass_kernel_spmd
from concourse.masks import make_identity

F16 = mybir.dt.float16
F32 = mybir.dt.float32
AF = mybir.ActivationFunctionType
ALU = mybir.AluOpType
AX = mybir.AxisListType

L = 200
B_TOTAL = 2048
N_CORES = 8
BC = B_TOTAL // N_CORES  # 256
HIN = 64
H = 256
G4 = 4 * H  # 1024
KC = 64  # clusters
BETA = 0.1

# gate order in torch reference: i, f, g, o (each H rows). We permute to i, f, o, g.
_PERM = np.concatenate(
    [np.arange(0, 2 * H), np.arange(3 * H, 4 * H), np.arange(2 * H, 3 * H)]
)


def _build(T=L, timing=False, repeat=1):
    nc = bacc.Bacc("TRN2", target_bir_lowering=False, debug=False)

    xt_d = nc.dram_tensor(
        "xt", [1 if timing else T, HIN + 1, BC], F16, kind="ExternalInput"
    )
    if not timing:
        w0k01_d = nc.dram_tensor("w0k01", [128, 2 * G4], F16, kind="ExternalInput")
        w0k2_d = nc.dram_tensor("w0k2", [HIN + 1, G4], F16, kind="ExternalInput")
        w1k_d = nc.dram_tensor("w1k", [128, 4 * G4], F16, kind="ExternalInput")
        b1r_d = nc.dram_tensor("b1r", [1, G4], F16, kind="ExternalInput")
        c2t_d = nc.dram_tensor("c2t", [128, 2 * KC], F32, kind="ExternalInput")
        cnorm_d = nc.dram_tensor("cnorm", [128, KC], F32, kind="ExternalInput")

    lat_d = nc.dram_tensor("lat", [BC, H], F32, kind="ExternalOutput")
    seg_d = nc.dram_tensor("seg", [KC, H + 2], F32, kind="ExternalOutput")

    with tile.TileContext(nc) as tc:
        with (
            tc.tile_pool(name="consts", bufs=1) as consts,
            tc.tile_pool(name="xtp", bufs=8) as xtp,
            tc.tile_pool(name="hp", bufs=3) as hp,
            tc.tile_pool(name="cp", bufs=2) as cp,
            tc.tile_pool(name="sifop", bufs=3) as sifop,
            tc.tile_pool(name="ewp", bufs=3) as ewp,
            tc.tile_pool(name="vqsb", bufs=2) as vqsb,
        ):
            # ---- constants / weights to SBUF ----
            w0k01 = consts.tile([128, 2 * G4], F16, tag="w0k01")
            w0k2 = consts.tile([HIN + 1, G4], F16, tag="w0k2")
            w1k = consts.tile([128, 4 * G4], F16, tag="w1k")
            b1r = consts.tile([1, G4], F16, tag="b1r")
            c2t = consts.tile([128, 2 * KC], F32, tag="c2t")
            cnorm = consts.tile([128, KC], F32, tag="cnorm")
            if timing:
                for tt in (w0k01, w0k2, w1k, b1r, c2t, cnorm):
                    nc.vector.memset(tt[:], 0.01)
            else:
                nc.sync.dma_start(w0k01[:], w0k01_d[:, :])
                nc.sync.dma_start(w0k2[:], w0k2_d[:, :])
                nc.sync.dma_start(w1k[:], w1k_d[:, :])
                nc.sync.dma_start(b1r[:], b1r_d[:, :])
                nc.sync.dma_start(c2t[:], c2t_d[:, :])
                nc.sync.dma_start(cnorm[:], cnorm_d[:, :])
            ones = consts.tile([1, BC], F16, tag="ones")
            nc.vector.memset(ones[:], 1.0)
            ident = consts.tile([128, 128], F32, tag="ident")
            make_identity(nc, ident[:])
            lat_sb = consts.tile([128, 2 * BC], F32, tag="lat_sb")

            # ---- initial states ----
            h0 = hp.tile([128, 2 * BC], F16, tag="h0")
            h1 = hp.tile([128, 2 * BC], F16, tag="h1")
            c0 = cp.tile([128, 2 * BC], F32, tag="c0")
            c1 = cp.tile([128, 2 * BC], F32, tag="c1")
            nc.vector.memset(h0[:], 0.0)
            nc.vector.memset(h1[:], 0.0)
            nc.vector.memset(c0[:], 0.0)
            nc.vector.memset(c1[:], 0.0)

            with (
                tc.tile_pool(name="g0psum", bufs=1, space="PSUM") as g0p,
                tc.tile_pool(name="g1psum", bufs=1, space="PSUM") as g1p,
            ):

                def ew_layer(g_ps, c_prev, h_out_f16, layer, also_f32=None):
                    """gate psum [128, 2048] (i|f|o|g blocks of 512) -> new c (f32), h."""
                    sig = sifop.tile([128, 3 * 2 * BC], F32, tag="sig")
                    nc.scalar.activation(sig[:], g_ps[:, 0 : 3 * 2 * BC], AF.Sigmoid)
                    tg = ewp.tile([128, 2 * BC], F32, tag="tg")
                    nc.scalar.activation(tg[:], g_ps[:, 3 * 2 * BC : 4 * 2 * BC], AF.Tanh)
                    fc = ewp.tile([128, 2 * BC], F32, tag="fc")
                    nc.vector.tensor_tensor(
                        fc[:], sig[:, 2 * BC : 4 * BC], c_prev[:], ALU.mult
                    )
                    ig = ewp.tile([128, 2 * BC], F32, tag="ig")
                    nc.vector.tensor_tensor(ig[:], sig[:, 0 : 2 * BC], tg[:], ALU.mult)
                    c_new = cp.tile([128, 2 * BC], F32, tag=f"c{layer}")
                    nc.vector.tensor_tensor(c_new[:], fc[:], ig[:], ALU.add)
                    tc_t = ewp.tile([128, 2 * BC], F32, tag="tc")
                    nc.scalar.activation(tc_t[:], c_new[:], AF.Tanh)
                    nc.vector.tensor_tensor(
                        h_out_f16[:], sig[:, 4 * BC : 6 * BC], tc_t[:], ALU.mult
                    )
                    if also_f32 is not None:
                        nc.vector.tensor_tensor(
                            also_f32[:], sig[:, 4 * BC : 6 * BC], tc_t[:], ALU.mult
                        )
                    return c_new

                for t in range(T * repeat):
                    xt = xtp.tile([HIN + 1, BC], F16, tag="xt")
                    nc.sync.dma_start(xt[:], xt_d[0 if timing else t])

                    # ---- layer 0 matmuls: gates0 = W_hh0 @ h0 + [W_ih0; b0] @ [x; 1]
                    g0 = g0p.tile([128, 8 * BC], F32, tag="g0")
                    for m in range(8):
                        ps = g0[:, BC * m : BC * (m + 1)]
                        nc.tensor.matmul(
                            ps,
                            w0k01[:, 128 * m : 128 * m + 128],
                            h0[:, 0:BC],
                            start=True,
                            stop=False,
                        )
                        nc.tensor.matmul(
                            ps,
                            w0k01[:, G4 + 128 * m : G4 + 128 * m + 128],
                            h0[:, BC : 2 * BC],
                            start=False,
                            stop=False,
                        )
                        nc.tensor.matmul(
                            ps,
                            w0k2[:, 128 * m : 128 * m + 128],
                            xt[:],
                            start=False,
                            stop=True,
                        )

                    h0n = hp.tile([128, 2 * BC], F16, tag="h0")
                    c0 = ew_layer(g0, c0, h0n, 0)

                    # ---- layer 1 matmuls: gates1 = b1 + W_hh1 @ h1 + W_ih1 @ h0n
                    g1 = g1p.tile([128, 8 * BC], F32, tag="g1")
                    for m in range(8):
                        ps = g1[:, BC * m : BC * (m + 1)]
                        nc.tensor.matmul(
                            ps,
                            b1r[0:1, 128 * m : 128 * m + 128],
                            ones[0:1, :],
                            start=True,
                            stop=False,
                        )
                        for k, mov in (
                            (0, h1[:, 0:BC]),
                            (1, h1[:, BC : 2 * BC]),
                            (2, h0n[:, 0:BC]),
                            (3, h0n[:, BC : 2 * BC]),
                        ):
                            nc.tensor.matmul(
                                ps,
                                w1k[:, G4 * k + 128 * m : G4 * k + 128 * m + 128],
                                mov,
                                start=False,
                                stop=(k == 3),
                            )

                    h1n = hp.tile([128, 2 * BC], F16, tag="h1")
                    c1 = ew_layer(
                        g1, c1, h1n, 1,
                        also_f32=lat_sb if t == T * repeat - 1 else None,
                    )
                    h0, h1 = h0n, h1n

            # ---- VQ tail ----
            with (
                tc.tile_pool(name="vqps", bufs=2, space="PSUM") as vqps,
                tc.tile_pool(name="segps", bufs=1, space="PSUM") as segps,
            ):
                seg_ps = segps.tile([KC, H + 2], F32, tag="segp")
                latb_tiles = []
                for b in range(2):
                    tp = vqps.tile([128, H], F32, tag="tp")
                    for j in range(2):
                        nc.tensor.transpose(
                            tp[:, 128 * j : 128 * j + 128],
                            lat_sb[:, BC * j + 128 * b : BC * j + 128 * b + 128],
                            ident[:],
                        )
                    latb = vqsb.tile([128, H + 2], F32, tag="latb")
                    latb_tiles.append(latb)
                    nc.vector.tensor_copy(latb[:, 0:H], tp[:])
                    nc.vector.memset(latb[:, H : H + 1], 1.0)
                    sq = vqsb.tile([128, H], F32, tag="sq")
                    nc.scalar.activation(
                        sq[:], tp[:], AF.Square, accum_out=latb[:, H + 1 : H + 2]
                    )
                    # D = lat @ (-2 C^T)  (+ ||c||^2)
                    dp = vqps.tile([128, KC], F32, tag="dp")
                    for j in range(2):
                        nc.tensor.matmul(
                            dp[:],
                            lat_sb[:, BC * j + 128 * b : BC * j + 128 * b + 128],
                            c2t[:, KC * j : KC * (j + 1)],
                            start=(j == 0),
                            stop=(j == 1),
                        )
                    dsb = vqsb.tile([128, KC], F32, tag="dsb")
                    nc.vector.tensor_tensor(dsb[:], dp[:], cnorm[:], ALU.add)
                    minv = vqsb.tile([128, 1], F32, tag="minv")
                    nc.vector.tensor_reduce(minv[:], dsb[:], axis=AX.X, op=ALU.min)
                    oh = vqsb.tile([128, KC], F32, tag="oh")
                    nc.vector.tensor_scalar(
                        oh[:], dsb[:], minv[:], None, op0=ALU.is_equal
                    )
                    nc.tensor.matmul(
                        seg_ps[:], oh[:], latb[:], start=(b == 0), stop=(b == 1)
                    )

                for b in range(2):
                    nc.sync.dma_start(
                        lat_d[128 * b : 128 * (b + 1), :], latb_tiles[b][:, 0:H]
                    )
                seg_sb = vqsb.tile([KC, H + 2], F32, tag="segsb")
                nc.vector.tensor_copy(seg_sb[:], seg_ps[:])
                nc.sync.dma_start(seg_d[:, :], seg_sb[:])

    nc.compile()
    return nc


_NC_CACHE = {}


def _get_nc(T=L):
    if T not in _NC_CACHE:
        _NC_CACHE[T] = _build(T)
    return _NC_CACHE[T]


def _prep_shared(W_ih0, W_hh0, b_ih0, b_hh0, W_ih1, W_hh1, b_ih1, b_hh1, clusters):
    p = _PERM
    w0k01 = (
        W_hh0[p].T.reshape(2, 128, G4).transpose(1, 0, 2).reshape(128, 2 * G4)
    ).astype(np.float16)
    w0k2 = np.concatenate(
        [W_ih0[p].T, (b_ih0 + b_hh0)[p][None, :]], axis=0
    ).astype(np.float16)
    wcat1 = np.concatenate([W_hh1[p].T, W_ih1[p].T], axis=0)  # (512, 1024)
    w1k = (
        wcat1.reshape(4, 128, G4).transpose(1, 0, 2).reshape(128, 4 * G4)
    ).astype(np.float16)
    b1r = (b_ih1 + b_hh1)[p][None, :].astype(np.float16)
    c2t = (
        (-2.0 * clusters).T.reshape(2, 128, KC).transpose(1, 0, 2).reshape(128, 2 * KC)
    ).astype(np.float32)
    cnorm = np.ascontiguousarray(
        np.broadcast_to((clusters * clusters).sum(1)[None, :], (128, KC))
    ).astype(np.float32)
    return dict(w0k01=w0k01, w0k2=w0k2, w1k=w1k, b1r=b1r, c2t=c2t, cnorm=cnorm)


def _prep_xt(X, c, T=L):
    xc = X[:T, c * BC : (c + 1) * BC, :]  # (T, BC, HIN)
    xt = np.empty((T, HIN + 1, BC), np.float16)
    xt[:, :HIN, :] = xc.transpose(0, 2, 1)
    xt[:, HIN, :] = 1.0
    return xt


def kernel(
    X,
    W_ih0,
    W_hh0,
    b_ih0,
    b_hh0,
    W_ih1,
    W_hh1,
    b_ih1,
    b_hh1,
    clusters,
    _T=L,
    _trace=False,
):
    nc = _get_nc(_T)
    shared = _prep_shared(
        W_ih0, W_hh0, b_ih0, b_hh0, W_ih1, W_hh1, b_ih1, b_hh1, clusters
    )
    in_maps = [dict(shared, xt=_prep_xt(X, c, _T)) for c in range(N_CORES)]
    res = run_bass_kernel_spmd(
        nc, in_maps, core_ids=list(range(N_CORES)), trace=_trace
    )

    latent = np.concatenate([r["lat"] for r in res.results], axis=0)
    seg = np.sum([r["seg"].astype(np.float64) for r in res.results], axis=0)

    sums = seg[:, :H]
    counts = seg[:, H]
    sumsq = seg[:, H + 1].sum()
    cl64 = clusters.astype(np.float64)
    upd = np.where(
        counts[:, None] > 0, sums / np.maximum(counts, 1.0)[:, None], cl64
    )
    loss = 0.5 * BETA * (
        sumsq - 2.0 * np.sum(upd * sums) + np.sum(counts * np.sum(upd * upd, axis=1))
    )
    out_loss = np.float32(loss)
    if _trace:
        kernel._last_results = res
    return latent, out_loss
